# revision 26
# baseline (speedup 1.0000x reference)
"""Trainium2 Bass kernel for a pre-LN transformer block (B=4, T=2048, C=1024, H=16).

Sharding over 8 cores: core c handles batch b=c//2 and head-group g=c%2
(8 of 16 heads). Each core computes LN1 + QKV + causal attention + its
partial head-slice of the output projection for ALL 2048 tokens of its
batch element, then a pair ReduceScatter sums the two partial projections
and hands each core a contiguous 1024-token half for residual + LN2 + FFN.

Matmul operands are bf16 (fp32 PSUM accumulation); the residual / layernorm
spine stays fp32. Softmax skips the max-subtraction (scores are O(1) here),
masks causal blocks additively, and gets row sums for free via a ones
column appended to V; normalization happens after P@V.

v2: the attention phase has NO dependency on the ReduceScatter outputs
(LN2 moved entirely into the FFN phase) so the Sync/DMA queue never
head-of-line blocks on a collective; softmax row normalization uses
gpsimd.partition_broadcast instead of DMA round-trips; FFN1 weights are
loaded once into SBUF (nh-outer loop); FFN2 is split by output halves so
PSUM fits and the first half drains early.
"""

import numpy as np
import ml_dtypes

import concourse.bass as bass
import concourse.mybir as mybir
import concourse.tile as tile
import bass_rust
from bass_rust import ScopedClock
from concourse.bass_utils import run_bass_kernel_spmd
from concourse.masks import make_identity

# ---------------------------------------------------------------------------
# Workaround: walrus in this toolchain rejects >1 sem wait on CTRL-queue
# instructions; split the final Tile drain's waits across single-wait nops.
_MAX_WAITS = 1


def _patched_drain_and_barrier(self, tick_clock, wait_clock):
    nc = self.nc
    probe = nc.sync.nop()
    wait_clock.add_sem_waits(probe.ins, ScopedClock({None: tick_clock.global_clock}))
    waits = list(probe.ins.sync_info.on_wait) if probe.ins.sync_info else []
    chunks = [waits[i:i + _MAX_WAITS] for i in range(0, len(waits), _MAX_WAITS)] or [[]]
    probe.ins.sync_info = bass_rust.SyncInfo(on_wait=chunks[0], on_update=[])
    for ch in chunks[1:]:
        n = nc.sync.nop()
        n.ins.sync_info = bass_rust.SyncInfo(on_wait=ch, on_update=[])
    nc.sync.drain()
    nc.all_engine_barrier()
    popped = nc._tile_sem_poison_stack.pop()
    assert popped is self._sem_poison
    nc.clear_and_free_semaphores(list(self.sems.allocated().values()))
    nc.all_engine_barrier()


tile.TileContext._drain_and_barrier = _patched_drain_and_barrier

# This walrus build accepts at most ONE sem wait on ANY instruction. Split
# multi-wait instructions at BIR-serialization time: excess waits move onto
# single-wait NoOps inserted immediately before, on the same engine.
import json as _json

_orig_to_json_bytes = bass.Bass.to_json_bytes


def _split_multi_waits_json(self) -> bytes:
    raw = _orig_to_json_bytes(self)
    j = _json.loads(raw)
    changed = False
    for func in j.get("functions", []):
        for blk in func.get("blocks", []):
            insts = blk.get("instructions", [])
            out = []
            for inst in insts:
                si = inst.get("sync_info")
                waits = si.get("on_wait") if si else None
                if waits and len(waits) > 1:
                    changed = True
                    for i, w in enumerate(waits[:-1]):
                        out.append({
                            "debug": inst.get("debug", 0),
                            "engine": inst["engine"],
                            "ins": [], "outs": [],
                            "name": f"{inst['name']}-sw{i}",
                            "opcode": "NoOp",
                            "sync_info": {"on_update": [], "on_wait": [w]},
                        })
                    si["on_wait"] = [waits[-1]]
                out.append(inst)
            blk["instructions"] = out
    if not changed:
        return raw
    return _json.dumps(j).encode()


bass.Bass.to_json_bytes = _split_multi_waits_json

# ---------------------------------------------------------------------------

N_CORES = 8
B, T, C = 4, 2048, 1024
H, HD = 16, 64
H_OWN = 8               # heads per core
HDIM_OWN = H_OWN * HD   # 512
F = 4 * C               # 4096
T_OWN = T // 2          # 1024 rows per core after reduce-scatter
LN_EPS = 1e-6
NT = T // 128           # 16 token tiles
SCALE = float(C) ** -0.5
MASK_VAL = -1e9

F32 = mybir.dt.float32
F32R = mybir.dt.float32r
BF16 = mybir.dt.bfloat16

_PROGRAM_CACHE = {}


def _build_program(has_b2, has_qkb):
    key = (has_b2, has_qkb)
    if key in _PROGRAM_CACHE:
        return _PROGRAM_CACHE[key]

    nc = bass.Bass("TRN2", target_bir_lowering=False, debug=False,
                   num_devices=N_CORES)

    x_d = nc.dram_tensor("x", [T, C], F32, kind="ExternalInput").ap()
    xh_d = nc.dram_tensor("xh", [T_OWN, C], F32, kind="ExternalInput").ap()
    wq_d = nc.dram_tensor("wq", [128, 8, HDIM_OWN], BF16, kind="ExternalInput").ap()
    wk_d = nc.dram_tensor("wk", [128, 8, HDIM_OWN], BF16, kind="ExternalInput").ap()
    wv_d = nc.dram_tensor("wv", [128, 8, HDIM_OWN], BF16, kind="ExternalInput").ap()
    wp_d = nc.dram_tensor("wproj", [128, 4, C], BF16, kind="ExternalInput").ap()
    w1_d = nc.dram_tensor("w1b", [F // 128, 128, 8, 128], BF16, kind="ExternalInput").ap()
    w2_d = nc.dram_tensor("w2", [F // 128, 128, C], BF16, kind="ExternalInput").ap()
    b1_d = nc.dram_tensor("b1r", [128, F // 128], F32, kind="ExternalInput").ap()
    b2_d = nc.dram_tensor("b2row", [1, C], BF16, kind="ExternalInput").ap()
    qb_d = nc.dram_tensor("qkvb", [128, 8], F32, kind="ExternalInput").ap()
    y_d = nc.dram_tensor("y", [T_OWN, C], F32, kind="ExternalOutput").ap()

    groups = [[0, 1], [2, 3], [4, 5], [6, 7]]
    fp = mybir.ActivationFunctionType

    with tile.TileContext(nc) as tc:
      with tc.tile_pool(name="dram", bufs=1, space="DRAM") as dram:
        prj0_d = dram.tile([T_OWN, C], BF16)
        prj1_d = dram.tile([T_OWN, C], BF16)
        rs_d = dram.tile([T_OWN, C], BF16)

        with tc.tile_pool(name="consts", bufs=1) as consts:
            ident = consts.tile([128, 128], F32)
            make_identity(nc, ident)
            ident_bf = consts.tile([128, 128], BF16)
            nc.vector.tensor_copy(ident_bf, ident)
            eps_t = consts.tile([128, 1], F32)
            nc.vector.memset(eps_t, LN_EPS)
            b1_sb = consts.tile([128, F // 128], F32)
            nc.sync.dma_start(out=b1_sb, in_=b1_d)
            ones_bf = consts.tile([128, 128], BF16)
            nc.vector.memset(ones_bf, 1.0)
            # lower-triangular keep-mask (q >= kt), so diagonal-tile masking
            # is a vector multiply instead of a gpsimd affine_select: the
            # gpsimd queue must stay empty in the attention phase or the
            # ReduceScatter instruction head-of-line blocks the mask ops.
            trilm = consts.tile([128, 128], BF16)
            nc.gpsimd.affine_select(
                out=trilm, in_=ones_bf,
                compare_op=mybir.AluOpType.is_ge,
                fill=0.0, base=0, pattern=[[1, 128]],
                channel_multiplier=-1)
            x_mid = consts.tile([128, 8, C], F32)
            if has_b2:
                b2_sb = consts.tile([1, C], BF16)
                nc.sync.dma_start(out=b2_sb, in_=b2_d)
            if has_qkb:
                qkvb_sb = consts.tile([128, 8], F32)
                nc.sync.dma_start(out=qkvb_sb, in_=qb_d)

            if True:
                # persist2: attention tensors, released before FFN
                with tc.tile_pool(name="persist2", bufs=1) as p2:
                    qT = p2.tile([128, 4, T], BF16)
                    kT = p2.tile([128, 4, T], BF16)
                    vtok = p2.tile([128, NT, 8 * 65], BF16)
                    attn = p2.tile([128, 4, T], BF16)

                    for _s in range(NT):
                        nc.vector.memset(
                            vtok.rearrange(
                                "p s (h e) -> p s h e", e=65)[:, _s, :, 64:65],
                            1.0)

                    # ---- phase 1: LN1 + transpose + QKV, per 512-chunk ----
                    with (nc.named_scope("p1_ln1qkv"),
                          tc.tile_pool(name="hTp", bufs=1) as hTp,
                          tc.tile_pool(name="wqkv", bufs=1) as wqkvp,
                          tc.tile_pool(name="ln1", bufs=2) as ln1p,
                          tc.tile_pool(name="ln1s", bufs=4) as ln1s,
                          tc.tile_pool(name="tp", bufs=3, space="PSUM") as tpp,
                          tc.tile_pool(name="qkps", bufs=4, space="PSUM") as qkp):
                        hT = hTp.tile([128, 8, T], BF16)
                        wq_sb = wqkvp.tile([128, 8, HDIM_OWN], BF16, tag="wq")
                        wk_sb = wqkvp.tile([128, 8, HDIM_OWN], BF16, tag="wk")
                        wv_sb = wqkvp.tile([128, 8, HDIM_OWN], BF16, tag="wv")
                        # first x tiles before the weights: the LN1+transpose
                        # pipeline only needs x, weights arrive during it
                        x_early = {}
                        for s in range(2):
                            xt = ln1p.tile([128, C], F32, tag="xt",
                                           name=f"xt_{s}")
                            nc.sync.dma_start(
                                out=xt, in_=x_d[s * 128:(s + 1) * 128, :])
                            x_early[s] = xt
                        nc.sync.dma_start(out=wv_sb, in_=wv_d)
                        nc.sync.dma_start(out=wq_sb, in_=wq_d)
                        nc.sync.dma_start(out=wk_sb, in_=wk_d)

                        for n in range(4):
                            for si in range(4):
                                s = 4 * n + si
                                if s in x_early:
                                    xt = x_early.pop(s)
                                else:
                                    xt = ln1p.tile([128, C], F32, tag="xt",
                                                   name=f"xt_{s}")
                                    nc.sync.dma_start(
                                        out=xt,
                                        in_=x_d[s * 128:(s + 1) * 128, :])
                                stats = ln1s.tile([128, 2, 6], F32, tag="stats",
                                                  name=f"st_{s}")
                                nc.vector.bn_stats(out=stats[:, 0, :],
                                                   in_=xt[:, 0:512])
                                nc.vector.bn_stats(out=stats[:, 1, :],
                                                   in_=xt[:, 512:1024])
                                mv = ln1s.tile([128, 2], F32, tag="mv",
                                               name=f"mv_{s}")
                                nc.vector.bn_aggr(out=mv, in_=stats)
                                rstd = ln1s.tile([128, 1], F32, tag="rstd",
                                                 name=f"rs_{s}")
                                nc.scalar.activation(out=rstd, in_=mv[:, 1:2],
                                                     func=fp.Sqrt, bias=eps_t,
                                                     scale=1.0)
                                nc.vector.reciprocal(out=rstd, in_=rstd)
                                ht = ln1p.tile([128, C], BF16, tag="ht",
                                               name=f"ht_{s}")
                                with nc.allow_low_precision(
                                        reason="matmul operand is bf16 anyway"):
                                    nc.vector.tensor_scalar(
                                        out=ht, in0=xt, scalar1=mv[:, 0:1],
                                        scalar2=rstd,
                                        op0=mybir.AluOpType.subtract,
                                        op1=mybir.AluOpType.mult)
                                for q in range(2):
                                    tp = tpp.tile([128, 512], BF16, tag="tp",
                                                  name=f"tp_{s}_{q}")
                                    for jj in range(4):
                                        cj = q * 4 + jj
                                        nc.tensor.transpose(
                                            out=tp[:, jj * 128:(jj + 1) * 128],
                                            in_=ht[:, cj * 128:(cj + 1) * 128],
                                            identity=ident_bf)
                                    nc.scalar.copy(
                                        hT[:, q * 4:(q + 1) * 4,
                                           s * 128:(s + 1) * 128],
                                        tp.rearrange("p (j t) -> p j t", j=4))
                                ps = qkp.tile([128, 512], F32, tag="v", bufs=2,
                                              name=f"vps_{s}")
                                for k in range(8):
                                    nc.tensor.matmul(
                                        ps, hT[:, k, s * 128:(s + 1) * 128],
                                        wv_sb[:, k, :],
                                        start=(k == 0), stop=(k == 7))
                                nc.scalar.copy(
                                    vtok.rearrange(
                                        "p s (h e) -> p s h e",
                                        e=65)[:, s, :, 0:64],
                                    ps.rearrange("p (h e) -> p h e", e=64))
                            for wsb, out_sb, boff in ((wq_sb, qT, 0),
                                                      (wk_sb, kT, 4)):
                                for m in range(4):
                                    ps = qkp.tile([128, 512], F32, tag="qk",
                                                  bufs=3,
                                                  name=f"qk_{n}_{boff}_{m}")
                                    for k in range(8):
                                        nc.tensor.matmul(
                                            ps, wsb[:, k, m * 128:(m + 1) * 128],
                                            hT[:, k, n * 512:(n + 1) * 512],
                                            start=(k == 0), stop=(k == 7))
                                    if has_qkb:
                                        nc.scalar.activation(
                                            out=out_sb[:, m,
                                                       n * 512:(n + 1) * 512],
                                            in_=ps, func=fp.Identity,
                                            bias=qkvb_sb[:, boff + m:
                                                         boff + m + 1])
                                    else:
                                        nc.scalar.copy(
                                            out_sb[:, m, n * 512:(n + 1) * 512],
                                            ps)

                    # ---- phase 3+4: causal attention + proj + split RS ----
                    with (nc.named_scope("p3_attn"),
                          tc.tile_pool(name="wprj", bufs=1) as wpp,
                          tc.tile_pool(name="sps", bufs=2, space="PSUM") as spsp,
                          tc.tile_pool(name="avps", bufs=2, space="PSUM") as avpsp,
                          tc.tile_pool(name="prjps", bufs=1, space="PSUM") as prjp,
                          tc.tile_pool(name="pexp", bufs=3) as pexp,
                          tc.tile_pool(name="prjsb", bufs=3) as prjsb,
                          tc.tile_pool(name="atmp", bufs=3) as atmp):
                        wp_sb = wpp.tile([128, 4, C], BF16)
                        nc.sync.dma_start(out=wp_sb, in_=wp_d)
                        for tch in (0, 2, 1, 3):
                            n_st = 4 * tch + 4
                            for hp in range(4):
                                pair = (2 * hp, 2 * hp + 1)
                                avs = {h: avpsp.tile([65, 512], F32, tag="av",
                                                     name=f"av_{tch}_{h}")
                                       for h in pair}
                                sps = {}

                                def emit_sp(j):
                                    lo = max(0, j - 4 * tch) * 128
                                    sp = spsp.tile([128, 1024], F32, tag="sp",
                                                   name=f"sp_{tch}_{hp}_{j}")
                                    for hi, h in enumerate(pair):
                                        mt, po = h // 2, (h % 2) * 64
                                        nc.tensor.matmul(
                                            sp[:, hi * 512 + lo:(hi + 1) * 512],
                                            kT[po:po + 64, mt,
                                               j * 128:(j + 1) * 128],
                                            qT[po:po + 64, mt,
                                               tch * 512 + lo:(tch + 1) * 512],
                                            start=True, stop=True)
                                    sps[j] = sp

                                def emit_consume(j):
                                    lo = max(0, j - 4 * tch) * 128
                                    sp = sps.pop(j)
                                    pe = pexp.tile([128, 1024], BF16, tag="pe",
                                                   name=f"pe_{tch}_{hp}_{j}")
                                    nc.scalar.activation(
                                        out=pe.rearrange(
                                            "p (g w) -> p g w", g=2)[:, :,
                                                                     lo:512],
                                        in_=sp.rearrange(
                                            "p (g w) -> p g w", g=2)[:, :,
                                                                     lo:512],
                                        func=fp.Exp, scale=SCALE)
                                    if j >= 4 * tch:
                                        for hi in range(2):
                                            nc.vector.tensor_mul(
                                                pe[:, hi * 512 + lo:
                                                   hi * 512 + lo + 128],
                                                pe[:, hi * 512 + lo:
                                                   hi * 512 + lo + 128],
                                                trilm)
                                    for hi, h in enumerate(pair):
                                        nc.tensor.matmul(
                                            avs[h][:, lo:512],
                                            vtok[:, j, h * 65:(h + 1) * 65],
                                            pe[:, hi * 512 + lo:(hi + 1) * 512],
                                            start=(j == 0),
                                            stop=(j == n_st - 1))

                                emit_sp(0)
                                if n_st > 1:
                                    emit_sp(1)
                                for j in range(n_st):
                                    if j + 2 < n_st:
                                        emit_sp(j + 2)
                                    emit_consume(j)
                                for h in pair:
                                    mt, po = h // 2, (h % 2) * 64
                                    av = avs[h]
                                    avsb = atmp.tile([65, 512], BF16,
                                                     tag="avsb",
                                                     name=f"avsb_{tch}_{h}")
                                    nc.vector.tensor_copy(avsb, av)
                                    rsc = atmp.tile([128, 4], BF16, tag="rsc",
                                                    name=f"rsc_{tch}_{h}")
                                    nc.sync.dma_start(
                                        out=rsc, in_=avsb[64:65, :])
                                    with nc.allow_low_precision(
                                            reason="softmax norm in bf16"):
                                        nc.vector.reciprocal(rsc, rsc)
                                    rsb = atmp.tile([65, 512], BF16, tag="rsb",
                                                    name=f"rsb_{tch}_{h}")
                                    nc.sync.dma_start(
                                        out=rsb[64:65, :], in_=rsc)
                                    rb = avpsp.tile([64, 512], F32, tag="rbp",
                                                    bufs=1,
                                                    name=f"rbp_{tch}_{h}")
                                    nc.tensor.matmul(
                                        rb, ones_bf[64:65, 0:64],
                                        rsb[64:65, :],
                                        start=True, stop=True)
                                    if po == 0:
                                        nc.vector.tensor_mul(
                                            attn[0:64, mt,
                                                 tch * 512:(tch + 1) * 512],
                                            avsb[0:64, :], rb)
                                    else:
                                        stg = atmp.tile([64, 512], BF16,
                                                        tag="stg",
                                                        name=f"stg_{tch}_{h}")
                                        nc.vector.tensor_mul(
                                            stg, avsb[0:64, :], rb)
                                        nc.sync.dma_start(
                                            out=attn[64:128, mt,
                                                     tch * 512:(tch + 1) * 512],
                                            in_=stg)
                            # proj for this chunk's 4 token tiles
                            prj_buf = prj0_d if tch % 2 == 0 else prj1_d
                            base = (tch // 2) * 512
                            for ti in range(4):
                                tm = 4 * tch + ti
                                ysb = prjsb.tile([128, C], BF16, tag="ysb",
                                                 name=f"ysb_{tm}")
                                for nh in range(2):
                                    ps = prjp.tile([128, 512], F32, tag="prj",
                                                   name=f"prj_{tm}_{nh}")
                                    for k in range(4):
                                        nc.tensor.matmul(
                                            ps,
                                            attn[:, k, tm * 128:(tm + 1) * 128],
                                            wp_sb[:, k,
                                                  nh * 512:(nh + 1) * 512],
                                            start=(k == 0), stop=(k == 3))
                                    nc.vector.tensor_copy(
                                        ysb[:, nh * 512:(nh + 1) * 512], ps)
                                nc.sync.dma_start(
                                    out=prj_buf[base + ti * 128:
                                                base + (ti + 1) * 128, :],
                                    in_=ysb)
                            if tch == 2:
                                with nc.named_scope("p4b_rs0"):
                                    nc.gpsimd.collective_compute(
                                        "ReduceScatter", mybir.AluOpType.add,
                                        replica_groups=groups,
                                        ins=[prj0_d.opt()],
                                        outs=[rs_d[0:512, :].opt()])
                                # prefetch the FFN residual base while the
                                # DMA queues are quiet
                                for s in range(8):
                                    nc.sync.dma_start(
                                        out=x_mid[:, s, :],
                                        in_=xh_d[s * 128:(s + 1) * 128, :])

                    with nc.named_scope("p4b_rs1"):
                        nc.gpsimd.collective_compute(
                            "ReduceScatter", mybir.AluOpType.add,
                            replica_groups=groups,
                            ins=[prj1_d.opt()], outs=[rs_d[512:1024, :].opt()])

                # persist2 (qT/kT/vtok/attn) released here
                # =================== FFN super-phase ===================
                with tc.tile_pool(name="ffn_persist", bufs=1) as fpers:
                    u_sb = fpers.tile([128, 32, T_OWN], BF16)
                    h2T = fpers.tile([128, 8, T_OWN], BF16)
                    ln2_mv = fpers.tile([128, 8, 2], F32)
                    ln2_rs = fpers.tile([128, 8, 1], F32)

                    with (nc.named_scope("p5_ln2ffn1"),
                          tc.tile_pool(name="ln2", bufs=3) as ln2p,
                          tc.tile_pool(name="ln2s", bufs=4) as ln2s,
                          tc.tile_pool(name="w1p", bufs=26) as w1p,
                          tc.tile_pool(name="tp2", bufs=3, space="PSUM") as tpp2,
                          tc.tile_pool(name="ups", bufs=4, space="PSUM") as upsp):
                        # --- DMAs ordered so a slow collective can never
                        # head-of-line block compute-critical loads ---
                        rst = {}
                        for s in range(4):
                            r = ln2p.tile([128, C], BF16, tag="rst", bufs=6,
                                          name=f"rst_{s}")
                            nc.sync.dma_start(
                                out=r, in_=rs_d[s * 128:(s + 1) * 128, :])
                            rst[s] = r
                        w1t = {}

                        def load_w1(m):
                            t = w1p.tile([128, 8, 128], BF16, tag="w1t",
                                         name=f"w1t_{m}")
                            nc.sync.dma_start(out=t, in_=w1_d[m])
                            w1t[m] = t

                        for m in range(26):
                            load_w1(m)
                        for s in range(4, 8):
                            r = ln2p.tile([128, C], BF16, tag="rst", bufs=6,
                                          name=f"rst_{s}")
                            nc.sync.dma_start(
                                out=r, in_=rs_d[s * 128:(s + 1) * 128, :])
                            rst[s] = r

                        def ln2_tile(s):
                            nc.vector.tensor_add(x_mid[:, s, :], x_mid[:, s, :],
                                                 rst.pop(s))
                            stats = ln2s.tile([128, 2, 6], F32, tag="stats2",
                                              name=f"st2_{s}")
                            nc.vector.bn_stats(out=stats[:, 0, :],
                                               in_=x_mid[:, s, 0:512])
                            nc.vector.bn_stats(out=stats[:, 1, :],
                                               in_=x_mid[:, s, 512:1024])
                            nc.vector.bn_aggr(out=ln2_mv[:, s, :], in_=stats)
                            nc.scalar.activation(
                                out=ln2_rs[:, s, :], in_=ln2_mv[:, s, 1:2],
                                func=fp.Sqrt, bias=eps_t, scale=1.0)
                            nc.vector.reciprocal(out=ln2_rs[:, s, :],
                                                 in_=ln2_rs[:, s, :])
                            h2 = ln2p.tile([128, C], BF16, tag="h2", bufs=2,
                                           name=f"h2_{s}")
                            with nc.allow_low_precision(
                                    reason="matmul operand is bf16 anyway"):
                                nc.vector.tensor_scalar(
                                    out=h2, in0=x_mid[:, s, :],
                                    scalar1=ln2_mv[:, s, 0:1],
                                    scalar2=ln2_rs[:, s, :],
                                    op0=mybir.AluOpType.subtract,
                                    op1=mybir.AluOpType.mult)
                            for q in range(2):
                                tp = tpp2.tile([128, 512], BF16, tag="tp2",
                                               name=f"tp2_{s}_{q}")
                                for jj in range(4):
                                    cj = q * 4 + jj
                                    nc.tensor.transpose(
                                        out=tp[:, jj * 128:(jj + 1) * 128],
                                        in_=h2[:, cj * 128:(cj + 1) * 128],
                                        identity=ident_bf)
                                nc.scalar.copy(
                                    h2T[:, q * 4:(q + 1) * 4,
                                        s * 128:(s + 1) * 128],
                                    tp.rearrange("p (j t) -> p j t", j=4))

                        def ffn1(m, nh):
                            ps = upsp.tile([128, 512], F32, tag="ups",
                                           name=f"ups_{nh}_{m}")
                            for k in range(8):
                                nc.tensor.matmul(
                                    ps, w1t[m][:, k],
                                    h2T[:, k, nh * 512:(nh + 1) * 512],
                                    start=(k == 0), stop=(k == 7))
                            nc.scalar.activation(
                                out=u_sb[:, m, nh * 512:(nh + 1) * 512],
                                in_=ps, func=fp.Relu,
                                bias=b1_sb[:, m:m + 1])

                        # half 0 over the first 24 m-tiles covers the RS1
                        # window; w1 tiles stay resident for the nh=1 pass
                        for si in range(4):
                            ln2_tile(si)
                        for m in range(26):
                            ffn1(m, 0)
                        for si in range(4, 8):
                            ln2_tile(si)
                        for m in range(26):
                            ffn1(m, 1)
                        for m in range(26, 32):
                            load_w1(m)
                            ffn1(m, 0)
                            ffn1(m, 1)

                    # ---- phase 6b: FFN2 + residual + out, split by output
                    # half so each half's PSUM drains early ----
                    with (nc.named_scope("p7_ffn2"),
                          tc.tile_pool(name="w2p", bufs=4) as w2p,
                          tc.tile_pool(name="yps", bufs=1, space="PSUM") as ypsp,
                          tc.tile_pool(name="outp", bufs=4) as outp):
                        for nh in range(2):
                            pss = [ypsp.tile([128, 512], F32, tag=f"yps{i}",
                                             name=f"yps_n{nh}_{i}")
                                   for i in range(8)]
                            for kk in range(16):
                                w2t = w2p.tile([128, 2, 512], BF16, tag="w2t",
                                               name=f"w2t_{nh}_{kk}")
                                nc.sync.dma_start(
                                    out=w2t,
                                    in_=w2_d[2 * kk:2 * kk + 2,
                                             :, nh * 512:(nh + 1) * 512
                                             ].rearrange("k p d -> p k d"))
                                for dk in range(2):
                                    k = 2 * kk + dk
                                    for i in range(8):
                                        nc.tensor.matmul(
                                            pss[i],
                                            u_sb[:, k, i * 128:(i + 1) * 128],
                                            w2t[:, dk],
                                            start=(k == 0),
                                            stop=(k == 31 and not has_b2))
                            for i in range(8):
                                if has_b2:
                                    nc.tensor.matmul(
                                        pss[i], ones_bf[0:1, :],
                                        b2_sb[:, nh * 512:(nh + 1) * 512],
                                        start=False, stop=True)
                                out_t = outp.tile([128, 512], F32, tag="out",
                                                  name=f"out_{nh}_{i}")
                                nc.vector.tensor_add(
                                    out_t,
                                    x_mid[:, i, nh * 512:(nh + 1) * 512],
                                    pss[i])
                                nc.sync.dma_start(
                                    out=y_d[i * 128:(i + 1) * 128,
                                            nh * 512:(nh + 1) * 512],
                                    in_=out_t)

    _PROGRAM_CACHE[key] = nc
    return nc


def _prep_inputs(inputs):
    """Host-side prep: LN-affine folding, head-group slicing, dtype casts."""
    bf = ml_dtypes.bfloat16
    x = np.asarray(inputs["x"], np.float32)
    wq = np.asarray(inputs["wq"], np.float32)
    wk = np.asarray(inputs["wk"], np.float32)
    wv = np.asarray(inputs["wv"], np.float32)
    w_proj = np.asarray(inputs["w_proj"], np.float32)
    b_proj = np.asarray(inputs["b_proj"], np.float32)
    s1 = np.asarray(inputs["ln1_scale"], np.float32)
    bb1 = np.asarray(inputs["ln1_bias"], np.float32)
    s2 = np.asarray(inputs["ln2_scale"], np.float32)
    bb2 = np.asarray(inputs["ln2_bias"], np.float32)
    w1 = np.asarray(inputs["w1"], np.float32)
    b1 = np.asarray(inputs["b1"], np.float32)
    w2 = np.asarray(inputs["w2"], np.float32)
    b2 = np.asarray(inputs["b2"], np.float32)

    # fold ln1 scale into QKV weights; ln1 bias becomes per-output bias
    Wq = (wq * s1[None, :, None]).transpose(1, 0, 2).reshape(C, H * HD)
    Wk = (wk * s1[None, :, None]).transpose(1, 0, 2).reshape(C, H * HD)
    Wv = (wv * s1[None, :, None]).transpose(1, 0, 2).reshape(C, H * HD)
    qb = bb1 @ Wq
    kb = bb1 @ Wk
    vb = bb1 @ Wv
    assert np.allclose(vb, 0.0, atol=1e-30), "nonzero ln1_bias@wv unsupported"

    w1_eff = w1 * s2[:, None]
    b1_eff = b1 + bb2 @ w1
    # [32 m, 128 p, 8 k, 128 d]: partition-contiguous DMA lines
    w1b = np.ascontiguousarray(
        w1_eff.reshape(8, 128, 32, 128).transpose(2, 1, 0, 3)).astype(bf)
    b1r = np.ascontiguousarray(b1_eff.reshape(32, 128).T).astype(np.float32)
    b2row = b2[None, :].astype(bf)
    has_b2 = bool(np.any(b2 != 0.0))
    has_qkb = bool(np.any(qb != 0.0) or np.any(kb != 0.0))
    w2_bf = np.ascontiguousarray(w2.reshape(32, 128, C)).astype(bf)

    def kmajor(w):  # [C, D] -> [128, 8, D] with C = k*128 + p
        D = w.shape[1]
        return np.ascontiguousarray(
            w.reshape(8, 128, D).transpose(1, 0, 2)).astype(bf)

    per_core = []
    for c in range(N_CORES):
        b, g = c // 2, c % 2
        sl = slice(g * HDIM_OWN, (g + 1) * HDIM_OWN)
        qkvb = np.zeros((128, 8), np.float32)
        for m in range(4):
            qkvb[:, m] = qb[sl][m * 128:(m + 1) * 128]
            qkvb[:, 4 + m] = kb[sl][m * 128:(m + 1) * 128]
        xh = x[b, g * T_OWN:(g + 1) * T_OWN, :] + b_proj[None, :]
        wp = np.ascontiguousarray(w_proj[sl, :]).reshape(4, 128, C)
        per_core.append({
            "x": np.ascontiguousarray(x[b]),
            "xh": np.ascontiguousarray(xh.astype(np.float32)),
            "wq": kmajor(Wq[:, sl]),
            "wk": kmajor(Wk[:, sl]),
            "wv": kmajor(Wv[:, sl]),
            "wproj": np.ascontiguousarray(
                wp.transpose(1, 0, 2)).astype(bf),
            "w1b": w1b,
            "w2": w2_bf,
            "b1r": b1r,
            "b2row": b2row,
            "qkvb": qkvb,
        })
    return per_core, has_b2, has_qkb


def _run(inputs, trace=False):
    per_core, has_b2, has_qkb = _prep_inputs(inputs)
    nc = _build_program(has_b2, has_qkb)
    res = run_bass_kernel_spmd(nc, per_core, core_ids=list(range(N_CORES)),
                               trace=trace)
    out = np.empty((B, T, C), np.float32)
    for c in range(N_CORES):
        b, g = c // 2, c % 2
        out[b, g * T_OWN:(g + 1) * T_OWN, :] = res.results[c]["y"]
    return out, res


def kernel(**inputs):
    out, _ = _run(inputs, trace=False)
    return out
